# revision 3
# baseline (speedup 1.0000x reference)
"""MeanFeatureGather (segment mean + gather) for Trainium2, 8 NeuronCores.

Sharding: 8 cores = 4 images (batch) x 2 half-images. Each core:
  Launch A: segment sums + counts of its half-image via the GPSIMD
            scatter_add ucode op (bf16, d=2 fused (feature, one) payload,
            32-way replica-slot rotation to defeat the ucode's
            read-modify-write pipeline hazard on duplicate indices),
            then a replica reduction (DVE) and a partition-combine
            matmul (PE) down to a [65, 800] (sums, counts) table.
  Host:     pairwise-sums the two half-image tables (shard combine).
  Launch B: divides to get per-segment means (DVE), then gathers the
            means back to all pixels channel-major via the GPSIMD
            ap_gather ucode op (f32, d=1), and streams the result out.
"""

import sys

sys.path.insert(0, "/opt/trn_rl_repo")

import numpy as np
import ml_dtypes

import concourse.bass as bass
import concourse.bacc as bacc
from concourse import mybir
from concourse.bass_utils import run_bass_kernel_spmd

B, C, N, K = 4, 64, 512 * 512, 400
NH = N // 2            # pixels per core (half image)        131072
R = 32                 # replica slots for scatter_add
D = 2                  # scatter payload: (feature, one)
NE = K * R             # table entries per partition          12800
JQ = NH // 2           # pixels per q7-core stream (2 blocks)  65536
CHUNK_A = 16384        # idx per scatter_add call
NCHUNK_A = JQ // CHUNK_A
CHUNK_B = 8192         # idx per ap_gather call
NCHUNK_B = JQ // CHUNK_B
MROWS = 65             # master rows (64 channels + 1 spare)

_CACHE = {}


def _build_phaseA():
    nc = bacc.Bacc("TRN2", target_bir_lowering=False, debug=False, num_devices=8)
    addv_d = nc.dram_tensor("addv", [128, JQ * D], mybir.dt.bfloat16, kind="ExternalInput")
    idxA_d = nc.dram_tensor("idxA", [128, JQ // 16], mybir.dt.int16, kind="ExternalInput")
    sel_d = nc.dram_tensor("sel", [128, MROWS], mybir.dt.bfloat16, kind="ExternalInput")
    master_d = nc.dram_tensor("master", [MROWS, 800], mybir.dt.float32, kind="ExternalOutput")

    sem = nc.alloc_semaphore("s")
    sp, gp, ve, pe, act = nc.sync, nc.gpsimd, nc.vector, nc.tensor, nc.scalar

    tbl = nc.alloc_sbuf_tensor("tbl", [128, NE * D], mybir.dt.bfloat16)
    sel_sb = nc.alloc_sbuf_tensor("sel_sb", [128, MROWS], mybir.dt.bfloat16)
    sumsf = nc.alloc_sbuf_tensor("sumsf", [128, 800], mybir.dt.float32)
    addv_sb = [nc.alloc_sbuf_tensor(f"addv{i}", [128, CHUNK_A * D], mybir.dt.bfloat16) for i in range(2)]
    idx_sb = [nc.alloc_sbuf_tensor(f"idx{i}", [128, CHUNK_A // 16], mybir.dt.int16) for i in range(2)]
    out_sb = nc.alloc_sbuf_tensor("out_sb", [MROWS, 800], mybir.dt.float32)

    nv = 0
    ve.memset(tbl[:], 0.0).then_inc(sem, 1); nv += 1
    sp.dma_start(sel_sb[:], sel_d[:]).then_inc(sem, 16); nv += 16
    for i in range(2):
        sp.dma_start(addv_sb[i][:], addv_d[:, i * CHUNK_A * D : (i + 1) * CHUNK_A * D]).then_inc(sem, 16); nv += 16
        sp.dma_start(idx_sb[i][:], idxA_d[:, i * (CHUNK_A // 16) : (i + 1) * (CHUNK_A // 16)]).then_inc(sem, 16); nv += 16

    gp.wait_ge(sem, nv)
    scat_done = nc.alloc_semaphore("scat")
    ns = 0
    for c in range(NCHUNK_A):
        buf = c % 2
        if c >= 2:
            sp.wait_ge(scat_done, ns - 1)
            sp.dma_start(addv_sb[buf][:], addv_d[:, c * CHUNK_A * D : (c + 1) * CHUNK_A * D]).then_inc(sem, 16); nv += 16
            sp.dma_start(idx_sb[buf][:], idxA_d[:, c * (CHUNK_A // 16) : (c + 1) * (CHUNK_A // 16)]).then_inc(sem, 16); nv += 16
            gp.wait_ge(sem, nv)
        gp.scatter_add(
            in_ap=tbl[:].rearrange("p (k d) -> p k d", d=D),
            idxs_ap=idx_sb[buf][:],
            add_ap=addv_sb[buf][:].rearrange("p (j d) -> p j d", d=D),
            channels=128, num_elems=NE, d=D, num_idxs=CHUNK_A,
        ).then_inc(scat_done, 1); ns += 1

    ve.wait_ge(scat_done, ns)
    ve.reduce_sum(
        sumsf[:],
        tbl[:].rearrange("p (r k e) -> p k e r", r=R, k=K, e=D)[:],
        axis=mybir.AxisListType.X,
    ).then_inc(sem, 1); nv += 1

    with (
        nc.psum_tensor([MROWS, 400], mybir.dt.float32) as ps0,
        nc.psum_tensor([MROWS, 400], mybir.dt.float32) as ps1,
    ):
        sumsf_bf = nc.alloc_sbuf_tensor("sumsf_bf", [128, 800], mybir.dt.bfloat16)
        ve.tensor_copy(sumsf_bf[:], sumsf[:]).then_inc(sem, 1); nv += 1
        pe.wait_ge(sem, nv)
        pe.matmul(ps0[:], sel_sb[:], sumsf_bf[:, 0:400], start=True, stop=True)
        pe.matmul(ps1[:], sel_sb[:], sumsf_bf[:, 400:800], start=True, stop=True).then_inc(sem, 1); nv += 1
        act.wait_ge(sem, nv)
        act.copy(out_sb[:, 0:400], ps0[:])
        act.copy(out_sb[:, 400:800], ps1[:]).then_inc(sem, 1); nv += 1
        sp.wait_ge(sem, nv)
        sp.dma_start(master_d[:], out_sb[:]).then_inc(sem, 16); nv += 16
        sp.wait_ge(sem, nv)
    nc.compile()
    return nc


def _build_phaseB():
    nc = bacc.Bacc("TRN2", target_bir_lowering=False, debug=False, num_devices=8)
    master_d = nc.dram_tensor("master", [MROWS, 800], mybir.dt.float32, kind="ExternalInput")
    idxB_d = nc.dram_tensor("idxB", [128, JQ // 16], mybir.dt.int16, kind="ExternalInput")
    out_d = nc.dram_tensor("outp", [C, NH], mybir.dt.float32, kind="ExternalOutput")

    sem = nc.alloc_semaphore("s")
    sp, gp, ve = nc.sync, nc.gpsimd, nc.vector

    ms = nc.alloc_sbuf_tensor("ms", [128, 800], mybir.dt.float32)       # sums/counts, duplicated
    means = nc.alloc_sbuf_tensor("means", [128, 400], mybir.dt.float32)  # gather table
    cnt = nc.alloc_sbuf_tensor("cnt", [128, 400], mybir.dt.float32)
    idxB_sb = nc.alloc_sbuf_tensor("idxB_sb", [128, JQ // 16], mybir.dt.int16)
    go_sb = [nc.alloc_sbuf_tensor(f"go{i}", [128, CHUNK_B], mybir.dt.float32) for i in range(2)]

    nv = 0
    sp.dma_start(ms[0:C, :], master_d[0:C, :]).then_inc(sem, 16); nv += 16
    sp.dma_start(ms[C:2 * C, :], master_d[0:C, :]).then_inc(sem, 16); nv += 16
    sp.dma_start(idxB_sb[:], idxB_d[:]).then_inc(sem, 16); nv += 16
    ve.wait_ge(sem, nv)
    # counts = max(counts, 1); means = sums / counts
    ve.tensor_scalar(out=cnt[:], in0=ms[:].rearrange("p (k e) -> p e k", e=2)[:, 1],
                     scalar1=1.0, scalar2=None, op0=mybir.AluOpType.max).then_inc(sem, 1); nv += 1
    ve.wait_ge(sem, nv)
    ve.reciprocal(cnt[:], cnt[:]).then_inc(sem, 1); nv += 1
    ve.wait_ge(sem, nv)
    ve.tensor_tensor(out=means[:], in0=ms[:].rearrange("p (k e) -> p e k", e=2)[:, 0],
                     in1=cnt[:], op=mybir.AluOpType.mult).then_inc(sem, 1); nv += 1

    gp.wait_ge(sem, nv)
    dve_base = nv
    gat_done = nc.alloc_semaphore("gat")
    ng = 0
    # out layout: partition p = b2*64 + c; two DMAs per chunk (one per b2)
    for c in range(NCHUNK_B):
        buf = c % 2
        if c >= 2:
            # both out-DMAs of chunk c-2 (same buffer) must have completed
            gp.wait_ge(sem, dve_base + (c - 1) * 32)
        gp.ap_gather(
            out_ap=go_sb[buf][:],
            in_ap=means[:],
            idxs_ap=idxB_sb[:, c * (CHUNK_B // 16) : (c + 1) * (CHUNK_B // 16)],
            channels=128, num_elems=400, d=1, num_idxs=CHUNK_B,
        ).then_inc(gat_done, 1); ng += 1
        sp.wait_ge(gat_done, ng)
        for b2 in range(2):
            sp.dma_start(
                out_d[:, b2 * JQ + c * CHUNK_B : b2 * JQ + (c + 1) * CHUNK_B],
                go_sb[buf][b2 * C : (b2 + 1) * C, :],
            ).then_inc(sem, 16)
    sp.wait_ge(sem, dve_base + NCHUNK_B * 32)
    nc.compile()
    return nc


def _get_ncs():
    if "A" not in _CACHE:
        _CACHE["A"] = _build_phaseA()
    if "B" not in _CACHE:
        _CACHE["B"] = _build_phaseB()
    return _CACHE["A"], _CACHE["B"]


_SEL = None


def _sel_matrix():
    global _SEL
    if _SEL is None:
        s = np.zeros((128, MROWS), dtype=ml_dtypes.bfloat16)
        for p in range(128):
            g = p // 16
            ch = (g // 2) * 16 + p % 16
            s[p, ch] = 1.0
        _SEL = s
    return _SEL


def _prep_A(feat_half, idx_half):
    """feat_half [64, NH] f32, idx_half [NH] int -> phase A inputs."""
    addv = np.empty((128, JQ, D), dtype=ml_dtypes.bfloat16)
    idxA = np.empty((128, JQ // 16), dtype=np.int16)
    slot = (np.arange(JQ) % R).astype(np.int64) * K
    fb = feat_half.astype(ml_dtypes.bfloat16)
    for g in range(8):
        sl, blk = g // 2, g % 2
        pix = slice(blk * JQ, (blk + 1) * JQ)
        addv[16 * g : 16 * g + 16, :, 0] = fb[sl * 16 : sl * 16 + 16, pix]
        addv[16 * g : 16 * g + 16, :, 1] = 1.0
        ie = (idx_half[pix] + slot).astype(np.int16)
        idxA[16 * g : 16 * g + 16] = ie.reshape(-1, 16).T
    return {"addv": addv.reshape(128, JQ * D), "idxA": idxA, "sel": _sel_matrix()}


def _prep_B(idx_half):
    idxB = np.empty((128, JQ // 16), dtype=np.int16)
    for g in range(8):
        b2 = g // 4
        pix = slice(b2 * JQ, (b2 + 1) * JQ)
        w = idx_half[pix].astype(np.int16).reshape(-1, 16).T
        idxB[16 * g : 16 * g + 16] = w
    return idxB


def kernel(features, spixel_idx):
    """features [4, 64, 262144] f32; spixel_idx [4, 262144] int -> [4, 64, 262144] f32."""
    features = np.asarray(features)
    spixel_idx = np.asarray(spixel_idx)
    ncA, ncB = _get_ncs()

    in_maps_A = []
    idx_halves = []
    for core in range(8):
        b, h = core // 2, core % 2
        feat_half = features[b][:, h * NH : (h + 1) * NH]
        idx_half = spixel_idx[b][h * NH : (h + 1) * NH].astype(np.int64)
        idx_halves.append(idx_half)
        in_maps_A.append(_prep_A(feat_half, idx_half))

    resA = run_bass_kernel_spmd(ncA, in_maps_A, core_ids=list(range(8)))
    masters = [resA.results[i]["master"] for i in range(8)]

    in_maps_B = []
    for core in range(8):
        b = core // 2
        comb = masters[2 * b] + masters[2 * b + 1]
        in_maps_B.append({"master": comb, "idxB": _prep_B(idx_halves[core])})

    resB = run_bass_kernel_spmd(ncB, in_maps_B, core_ids=list(range(8)))

    out = np.empty((B, C, N), dtype=np.float32)
    for core in range(8):
        b, h = core // 2, core % 2
        out[b][:, h * NH : (h + 1) * NH] = resB.results[core]["outp"]
    return out


# revision 8
# speedup vs baseline: 2.1187x; 2.1187x over previous
"""MeanFeatureGather (per-segment mean + gather back) on 8 Trainium2 NeuronCores.

Sharding: 8 cores = 4 images (batch) x 2 half-images; each half-image is
processed channel-pair-major: SBUF partition p covers channel pair
a(p) = (p//64)*16 + p%16 and pixel block b(p) = (p//16)%4 (quarter of the
half-image), so all 8 GPSIMD Q7 cores work in parallel.

Launch A (per core): segment sums via the GPSIMD scatter_add ucode op
  (bf16, d=2 channel-pair payload, 32-way replica-slot rotation to defeat
  the ucode's pipelined read-modify-write hazard on duplicate indices),
  then a separate ones-payload scatter pass for the counts, DVE replica
  reductions, and a PE matmul that collapses partitions into a small
  [64, 1600] (sums, counts) table per core.
Host: pairwise adds the two half-image tables of each image (shard combine).
Launch B (per core): divides to per-segment means (DVE), packs an fp16
  channel-pair gather table, and gathers means to all pixels with the
  GPSIMD ap_gather ucode op (fp16, d=2 -> both channels of a pair per
  index), streaming fp16 results out; the host unpacks to [B, C, N] f32.
"""

import sys

sys.path.insert(0, "/opt/trn_rl_repo")

import numpy as np
import ml_dtypes

import concourse.bass as bass
import concourse.bacc as bacc
from concourse import mybir
from concourse.bass_utils import run_bass_kernel_spmd

B, C, N, K = 4, 64, 512 * 512, 400
NH = N // 2              # pixels per core (half image)          131072
R = 32                   # replica slots (scatter hazard window)
NE = K * R               # scatter table entries per partition    12800
NPAIR = C // 2           # channel pairs                          32
NBLK = 4                 # pixel blocks per half-image
JQ = NH // NBLK          # pixels per q7-core stream              32768
CHUNK_A = 8192           # idx per scatter_add call
NCHUNK_A = JQ // CHUNK_A # 4
CHUNK_B = 8192           # idx per ap_gather call
NCHUNK_B = JQ // CHUNK_B # 4

_CACHE = {}
LAST_HW_NS = None

_BF16 = ml_dtypes.bfloat16
_FP16 = np.float16


def _pal(p):
    """partition -> (pair a, block b). g = p//16: a = (g//4)*16 + p%16, b = g%4."""
    g = p // 16
    return (g // 4) * 16 + p % 16, g % 4


def _build_phaseA():
    nc = bacc.Bacc("TRN2", target_bir_lowering=False, debug=False, num_devices=8)
    addv_d = nc.dram_tensor("addv", [128, JQ * 2], mybir.dt.bfloat16, kind="ExternalInput")
    idxA_d = nc.dram_tensor("idxA", [128, JQ // 16], mybir.dt.int16, kind="ExternalInput")
    sel_d = nc.dram_tensor("sel", [128, NPAIR], mybir.dt.bfloat16, kind="ExternalInput")
    master_d = nc.dram_tensor("master", [C, 1600], mybir.dt.float32, kind="ExternalOutput")

    sem = nc.alloc_semaphore("s")
    sp, gp, ve, pe, act = nc.sync, nc.gpsimd, nc.vector, nc.tensor, nc.scalar

    tbl = nc.alloc_sbuf_tensor("tbl", [128, NE * 2], mybir.dt.bfloat16)       # 51.2 KB
    sel_sb = nc.alloc_sbuf_tensor("sel_sb", [128, NPAIR], mybir.dt.bfloat16)
    idxA_sb = nc.alloc_sbuf_tensor("idxA_sb", [128, JQ // 16], mybir.dt.int16)  # 4 KB
    addv_sb = [nc.alloc_sbuf_tensor(f"addv{i}", [128, CHUNK_A * 2], mybir.dt.bfloat16) for i in range(2)]  # 32 KB ea
    ones_sb = nc.alloc_sbuf_tensor("ones_sb", [128, CHUNK_A * 2], mybir.dt.bfloat16)  # 32 KB
    sumsf = nc.alloc_sbuf_tensor("sumsf", [128, 800], mybir.dt.float32)
    cntf = nc.alloc_sbuf_tensor("cntf", [128, 800], mybir.dt.float32)
    red_bf = nc.alloc_sbuf_tensor("red_bf", [128, 800], mybir.dt.bfloat16)
    out_sb = nc.alloc_sbuf_tensor("out_sb", [C, 1600], mybir.dt.float32)

    nv = 0
    ve.memset(tbl[:], 0.0)
    ve.memset(ones_sb[:], 1.0).then_inc(sem, 1); nv += 1
    sp.dma_start(sel_sb[:], sel_d[:]).then_inc(sem, 16); nv += 16
    sp.dma_start(idxA_sb[:], idxA_d[:]).then_inc(sem, 16); nv += 16
    for i in range(2):
        sp.dma_start(addv_sb[i][:], addv_d[:, i * CHUNK_A * 2 : (i + 1) * CHUNK_A * 2]).then_inc(sem, 16); nv += 16

    scat = nc.alloc_semaphore("scat")
    ns = 0
    gp.wait_ge(sem, nv)
    # ---- feature scatter (channel pairs) ----
    for cidx in range(NCHUNK_A):
        buf = cidx % 2
        if cidx >= 2:
            sp.wait_ge(scat, ns - 1)
            sp.dma_start(addv_sb[buf][:], addv_d[:, cidx * CHUNK_A * 2 : (cidx + 1) * CHUNK_A * 2]).then_inc(sem, 16); nv += 16
            gp.wait_ge(sem, nv)
        gp.scatter_add(
            in_ap=tbl[:].rearrange("p (k e) -> p k e", e=2),
            idxs_ap=idxA_sb[:, cidx * (CHUNK_A // 16) : (cidx + 1) * (CHUNK_A // 16)],
            add_ap=addv_sb[buf][:].rearrange("p (j e) -> p j e", e=2),
            channels=128, num_elems=NE, d=2, num_idxs=CHUNK_A,
        ).then_inc(scat, 1); ns += 1

    # ---- reduce feature sums: [p, (r k e)] -> [p, (k e)] over r ----
    ve.wait_ge(scat, ns)
    ve.reduce_sum(
        sumsf[:],
        tbl[:].rearrange("p (r k e) -> p k e r", r=R, k=K, e=2)[:],
        axis=mybir.AxisListType.X,
    ).then_inc(sem, 1); nv += 1

    # ---- re-zero table (DVE, after its reduce), then counts scatter with ones ----
    ve.memset(tbl[:], 0.0).then_inc(sem, 1); nv += 1
    gp.wait_ge(sem, nv)
    for cidx in range(NCHUNK_A):
        gp.scatter_add(
            in_ap=tbl[:].rearrange("p (k e) -> p k e", e=2),
            idxs_ap=idxA_sb[:, cidx * (CHUNK_A // 16) : (cidx + 1) * (CHUNK_A // 16)],
            add_ap=ones_sb[:].rearrange("p (j e) -> p j e", e=2),
            channels=128, num_elems=NE, d=2, num_idxs=CHUNK_A,
        ).then_inc(scat, 1); ns += 1
    ve.wait_ge(scat, ns)
    ve.reduce_sum(
        cntf[:],
        tbl[:].rearrange("p (r k e) -> p k e r", r=R, k=K, e=2)[:],
        axis=mybir.AxisListType.X,
    ).then_inc(sem, 1); nv += 1

    # ---- collapse partitions with PE: master = sel.T @ {sums, counts} ----
    with (
        nc.psum_tensor([NPAIR, 400], mybir.dt.float32) as ps0,
        nc.psum_tensor([NPAIR, 400], mybir.dt.float32) as ps1,
    ):
        for half, src in ((0, sumsf), (1, cntf)):
            ve.wait_ge(sem, nv)
            ve.tensor_copy(red_bf[:], src[:]).then_inc(sem, 1); nv += 1
            pe.wait_ge(sem, nv)
            pe.matmul(ps0[:], sel_sb[:], red_bf[:, 0:400], start=True, stop=True)
            pe.matmul(ps1[:], sel_sb[:], red_bf[:, 400:800], start=True, stop=True).then_inc(sem, 1); nv += 1
            act.wait_ge(sem, nv)
            act.copy(out_sb[0:NPAIR, half * 800 : half * 800 + 400], ps0[:])
            act.copy(out_sb[0:NPAIR, half * 800 + 400 : half * 800 + 800], ps1[:]).then_inc(sem, 1); nv += 1
        sp.wait_ge(sem, nv)
        sp.dma_start(master_d[0:NPAIR, :], out_sb[0:NPAIR, :]).then_inc(sem, 16); nv += 16
        sp.wait_ge(sem, nv)
    nc.compile()
    return nc


def _build_phaseB():
    nc = bacc.Bacc("TRN2", target_bir_lowering=False, debug=False, num_devices=8)
    # sums/cnt ship pair-interleaved: row a, col 2k+e = value for channel 2a+e
    sums_d = nc.dram_tensor("sums", [NPAIR, 800], mybir.dt.float32, kind="ExternalInput")
    cnt_d = nc.dram_tensor("cnt", [NPAIR, 800], mybir.dt.float32, kind="ExternalInput")
    idxB_d = nc.dram_tensor("idxB", [128, JQ // 16], mybir.dt.int16, kind="ExternalInput")
    out_d = nc.dram_tensor("outp", [128, JQ * 2], mybir.dt.float16, kind="ExternalOutput")
    mscr_d = nc.dram_tensor("mscr", [NPAIR, 800], mybir.dt.float16)  # internal scratch

    sem = nc.alloc_semaphore("s")
    sp, gp, ve = nc.sync, nc.gpsimd, nc.vector

    sums_sb = nc.alloc_sbuf_tensor("sums_sb", [NPAIR, 800], mybir.dt.float32)
    cnt_sb = nc.alloc_sbuf_tensor("cnt_sb", [NPAIR, 800], mybir.dt.float32)
    means16 = nc.alloc_sbuf_tensor("means16", [NPAIR, 800], mybir.dt.float16)
    tblB = nc.alloc_sbuf_tensor("tblB", [128, 800], mybir.dt.float16)
    idxB_sb = nc.alloc_sbuf_tensor("idxB_sb", [128, JQ // 16], mybir.dt.int16)
    go_sb = [nc.alloc_sbuf_tensor(f"go{i}", [128, CHUNK_B * 2], mybir.dt.float16) for i in range(2)]

    nv = 0
    sp.dma_start(sums_sb[:], sums_d[:]).then_inc(sem, 16); nv += 16
    sp.dma_start(cnt_sb[:], cnt_d[:]).then_inc(sem, 16); nv += 16
    sp.dma_start(idxB_sb[:], idxB_d[:]).then_inc(sem, 16); nv += 16
    ve.wait_ge(sem, nv)
    ve.tensor_scalar(out=cnt_sb[:], in0=cnt_sb[:], scalar1=1.0, scalar2=None,
                     op0=mybir.AluOpType.max).then_inc(sem, 1); nv += 1
    ve.wait_ge(sem, nv)
    ve.reciprocal(cnt_sb[:], cnt_sb[:]).then_inc(sem, 1); nv += 1
    ve.wait_ge(sem, nv)
    ve.tensor_tensor(out=sums_sb[:], in0=sums_sb[:], in1=cnt_sb[:],
                     op=mybir.AluOpType.mult).then_inc(sem, 1); nv += 1
    ve.wait_ge(sem, nv)
    ve.tensor_copy(means16[:], sums_sb[:]).then_inc(sem, 1); nv += 1
    sp.wait_ge(sem, nv)
    sp.dma_start(mscr_d[:], means16[:]).then_inc(sem, 16); nv += 16
    # build the pair table: tblB[p=(s,b,c16), (k e)] = mscr[s*16+c16, (k e)]
    sp.wait_ge(sem, nv)
    for g in range(8):
        s = g // 4
        sp.dma_start(
            tblB[16 * g : 16 * g + 16, :],
            mscr_d[16 * s : 16 * s + 16, :],
        ).then_inc(sem, 16); nv += 16

    gp.wait_ge(sem, nv)
    base = nv
    gat = nc.alloc_semaphore("gat")
    ng = 0
    for cidx in range(NCHUNK_B):
        buf = cidx % 2
        if cidx >= 2:
            gp.wait_ge(sem, base + (cidx - 1) * 16)
        gp.ap_gather(
            out_ap=go_sb[buf][:].rearrange("p (j e) -> p j e", e=2),
            in_ap=tblB[:].rearrange("p (k e) -> p k e", e=2),
            idxs_ap=idxB_sb[:, cidx * (CHUNK_B // 16) : (cidx + 1) * (CHUNK_B // 16)],
            channels=128, num_elems=400, d=2, num_idxs=CHUNK_B,
        ).then_inc(gat, 1); ng += 1
        sp.wait_ge(gat, ng)
        sp.dma_start(out_d[:, cidx * CHUNK_B * 2 : (cidx + 1) * CHUNK_B * 2], go_sb[buf][:]).then_inc(sem, 16)
    sp.wait_ge(sem, base + NCHUNK_B * 16)
    nc.compile()
    return nc


def _get_ncs():
    if "A" not in _CACHE:
        _CACHE["A"] = _build_phaseA()
    if "B" not in _CACHE:
        _CACHE["B"] = _build_phaseB()
    return _CACHE["A"], _CACHE["B"]


_SEL = None


def _sel_matrix():
    global _SEL
    if _SEL is None:
        s = np.zeros((128, NPAIR), dtype=_BF16)
        for p in range(128):
            a, _ = _pal(p)
            s[p, a] = 1.0
        _SEL = s
    return _SEL


_SLOT = None


def _slot_offsets():
    global _SLOT
    if _SLOT is None:
        _SLOT = ((np.arange(JQ) % R) * K).astype(np.int64)
    return _SLOT


def _prep_A(feat_half, idx_half):
    """feat_half [64, NH] f32, idx_half [NH] -> phase A inputs."""
    # partition p = (s, b, c16): a = s*16 + c16; channel = 2a + e
    addv = np.empty((2, NBLK, 16, JQ, 2), dtype=_BF16)  # [s, b, c16, j, e]
    fr = feat_half.astype(_BF16).reshape(2, 16, 2, NBLK, JQ)  # [s, c16, e, b, j]
    addv[:] = fr.transpose(0, 3, 1, 4, 2)  # -> [s, b, c16, j, e]
    idxw = np.empty((2, NBLK, 16, JQ // 16), dtype=np.int16)
    slot = _slot_offsets()
    for b in range(NBLK):
        ie = (idx_half[b * JQ : (b + 1) * JQ] + slot).astype(np.int16)
        w = ie.reshape(-1, 16).T  # [16, JQ//16]
        idxw[0, b] = w
        idxw[1, b] = w
    return {
        "addv": addv.reshape(128, JQ * 2),
        "idxA": idxw.reshape(128, JQ // 16),
        "sel": _sel_matrix(),
    }


def _prep_B(idx_half):
    idxw = np.empty((2, NBLK, 16, JQ // 16), dtype=np.int16)
    for b in range(NBLK):
        w = idx_half[b * JQ : (b + 1) * JQ].astype(np.int16).reshape(-1, 16).T
        idxw[0, b] = w
        idxw[1, b] = w
    return idxw.reshape(128, JQ // 16)


def _unpack_master(master):
    """[64, 1600] -> (sums_pair [32, 800] f32, counts [400] f32)."""
    return master[0:NPAIR, 0:800], master[0, 800:1600].reshape(400, 2)[:, 0]


def _unpack_out(buf):
    """[128, JQ*2] fp16 -> [64, NH] f32."""
    v = buf.reshape(2, NBLK, 16, JQ, 2)          # [s, b, c16, j, e]
    v = v.transpose(0, 2, 4, 1, 3)               # [s, c16, e, b, j]
    return v.reshape(C, NH).astype(np.float32)


def kernel(features, spixel_idx):
    """features [4, 64, 262144] f32; spixel_idx [4, 262144] int -> [4, 64, 262144] f32."""
    global LAST_HW_NS
    import time as _time

    features = np.asarray(features)
    spixel_idx = np.asarray(spixel_idx)
    ncA, ncB = _get_ncs()

    in_maps_A = []
    idx_halves = []
    for core in range(8):
        b, h = core // 2, core % 2
        feat_half = features[b][:, h * NH : (h + 1) * NH]
        idx_half = np.asarray(spixel_idx[b][h * NH : (h + 1) * NH], dtype=np.int64)
        idx_halves.append(idx_half)
        in_maps_A.append(_prep_A(feat_half, idx_half))

    t0 = _time.time()
    resA = run_bass_kernel_spmd(ncA, in_maps_A, core_ids=list(range(8)))
    tA = _time.time() - t0

    in_maps_B = []
    for core in range(8):
        b = core // 2
        s0, c0 = _unpack_master(resA.results[2 * b]["master"])
        s1, c1 = _unpack_master(resA.results[2 * b + 1]["master"])
        sums = np.ascontiguousarray(s0 + s1)
        counts = c0 + c1
        cnt_pair = np.ascontiguousarray(
            np.broadcast_to(np.repeat(counts, 2)[None, :], (NPAIR, 800))
        ).astype(np.float32)
        in_maps_B.append({
            "sums": sums,
            "cnt": cnt_pair,
            "idxB": _prep_B(idx_halves[core]),
        })

    t1 = _time.time()
    resB = run_bass_kernel_spmd(ncB, in_maps_B, core_ids=list(range(8)))
    tB = _time.time() - t1
    LAST_HW_NS = int((tA + tB) * 1e9)

    out = np.empty((B, C, N), dtype=np.float32)
    for core in range(8):
        b, h = core // 2, core % 2
        out[b][:, h * NH : (h + 1) * NH] = _unpack_out(resB.results[core]["outp"])
    return out


# revision 9
# speedup vs baseline: 2.1891x; 1.0332x over previous
"""MeanFeatureGather (per-segment mean + gather back) on 8 Trainium2 NeuronCores.

Sharding: 8 cores = 4 images (batch) x 2 half-images; each half-image is
processed channel-pair-major: SBUF partition p covers channel pair
a(p) = (p//64)*16 + p%16 and pixel block b(p) = (p//16)%4 (quarter of the
half-image), so all 8 GPSIMD Q7 cores work in parallel.

Launch A (per core): segment sums via the GPSIMD scatter_add ucode op
  (bf16, d=2 channel-pair payload, 32-way replica-slot rotation to defeat
  the ucode's pipelined read-modify-write hazard on duplicate indices),
  then a separate ones-payload scatter pass for the counts, DVE replica
  reductions, and a PE matmul that collapses partitions into a small
  [64, 1600] (sums, counts) table per core.
Host: pairwise adds the two half-image tables of each image (shard combine).
Launch B (per core): divides to per-segment means (DVE), packs an fp16
  channel-pair gather table, and gathers means to all pixels with the
  GPSIMD ap_gather ucode op (fp16, d=2 -> both channels of a pair per
  index), streaming fp16 results out; the host unpacks to [B, C, N] f32.
"""

import sys

sys.path.insert(0, "/opt/trn_rl_repo")

import numpy as np
import ml_dtypes

import concourse.bass as bass
import concourse.bacc as bacc
from concourse import mybir
from concourse.bass_utils import run_bass_kernel_spmd

B, C, N, K = 4, 64, 512 * 512, 400
NH = N // 2              # pixels per core (half image)          131072
R = 32                   # replica slots (scatter hazard window)
NE = K * R               # scatter table entries per partition    12800
NPAIR = C // 2           # channel pairs                          32
NBLK = 4                 # pixel blocks per half-image
JQ = NH // NBLK          # pixels per q7-core stream              32768
CHUNK_A = 8192           # idx per scatter_add call
NCHUNK_A = JQ // CHUNK_A # 4
CHUNK_B = 8192           # idx per ap_gather call
NQUAD = C // 4           # channel quads (phase B)                 16
JQ8 = NH // 8            # pixels per q7 stream in phase B         16384
NCHUNK_B8 = JQ8 // CHUNK_B  # 2

_CACHE = {}
LAST_HW_NS = None

_BF16 = ml_dtypes.bfloat16
_FP16 = np.float16


def _pal(p):
    """partition -> (pair a, block b). g = p//16: a = (g//4)*16 + p%16, b = g%4."""
    g = p // 16
    return (g // 4) * 16 + p % 16, g % 4


def _build_phaseA():
    nc = bacc.Bacc("TRN2", target_bir_lowering=False, debug=False, num_devices=8)
    addv_d = nc.dram_tensor("addv", [128, JQ * 2], mybir.dt.bfloat16, kind="ExternalInput")
    idxA_d = nc.dram_tensor("idxA", [128, JQ // 16], mybir.dt.int16, kind="ExternalInput")
    sel_d = nc.dram_tensor("sel", [128, NPAIR], mybir.dt.bfloat16, kind="ExternalInput")
    master_d = nc.dram_tensor("master", [C, 1600], mybir.dt.float32, kind="ExternalOutput")

    sem = nc.alloc_semaphore("s")
    sp, gp, ve, pe, act = nc.sync, nc.gpsimd, nc.vector, nc.tensor, nc.scalar

    tbl = nc.alloc_sbuf_tensor("tbl", [128, NE * 2], mybir.dt.bfloat16)       # 51.2 KB
    sel_sb = nc.alloc_sbuf_tensor("sel_sb", [128, NPAIR], mybir.dt.bfloat16)
    idxA_sb = nc.alloc_sbuf_tensor("idxA_sb", [128, JQ // 16], mybir.dt.int16)  # 4 KB
    addv_sb = [nc.alloc_sbuf_tensor(f"addv{i}", [128, CHUNK_A * 2], mybir.dt.bfloat16) for i in range(2)]  # 32 KB ea
    ones_sb = nc.alloc_sbuf_tensor("ones_sb", [128, CHUNK_A * 2], mybir.dt.bfloat16)  # 32 KB
    sumsf = nc.alloc_sbuf_tensor("sumsf", [128, 800], mybir.dt.float32)
    cntf = nc.alloc_sbuf_tensor("cntf", [128, 800], mybir.dt.float32)
    red_bf = nc.alloc_sbuf_tensor("red_bf", [128, 800], mybir.dt.bfloat16)
    out_sb = nc.alloc_sbuf_tensor("out_sb", [C, 1600], mybir.dt.float32)

    nv = 0
    ve.memset(tbl[:], 0.0)
    ve.memset(ones_sb[:], 1.0).then_inc(sem, 1); nv += 1
    sp.dma_start(sel_sb[:], sel_d[:]).then_inc(sem, 16); nv += 16
    sp.dma_start(idxA_sb[:], idxA_d[:]).then_inc(sem, 16); nv += 16
    for i in range(2):
        sp.dma_start(addv_sb[i][:], addv_d[:, i * CHUNK_A * 2 : (i + 1) * CHUNK_A * 2]).then_inc(sem, 16); nv += 16

    scat = nc.alloc_semaphore("scat")
    ns = 0
    gp.wait_ge(sem, nv)
    # ---- feature scatter (channel pairs) ----
    for cidx in range(NCHUNK_A):
        buf = cidx % 2
        if cidx >= 2:
            sp.wait_ge(scat, ns - 1)
            sp.dma_start(addv_sb[buf][:], addv_d[:, cidx * CHUNK_A * 2 : (cidx + 1) * CHUNK_A * 2]).then_inc(sem, 16); nv += 16
            gp.wait_ge(sem, nv)
        gp.scatter_add(
            in_ap=tbl[:].rearrange("p (k e) -> p k e", e=2),
            idxs_ap=idxA_sb[:, cidx * (CHUNK_A // 16) : (cidx + 1) * (CHUNK_A // 16)],
            add_ap=addv_sb[buf][:].rearrange("p (j e) -> p j e", e=2),
            channels=128, num_elems=NE, d=2, num_idxs=CHUNK_A,
        ).then_inc(scat, 1); ns += 1

    # ---- reduce feature sums: [p, (r k e)] -> [p, (k e)] over r ----
    ve.wait_ge(scat, ns)
    ve.reduce_sum(
        sumsf[:],
        tbl[:].rearrange("p (r k e) -> p k e r", r=R, k=K, e=2)[:],
        axis=mybir.AxisListType.X,
    ).then_inc(sem, 1); nv += 1

    # ---- re-zero table (DVE, after its reduce), then counts scatter with ones ----
    ve.memset(tbl[:], 0.0).then_inc(sem, 1); nv += 1
    gp.wait_ge(sem, nv)
    for cidx in range(NCHUNK_A):
        gp.scatter_add(
            in_ap=tbl[:].rearrange("p (k e) -> p k e", e=2),
            idxs_ap=idxA_sb[:, cidx * (CHUNK_A // 16) : (cidx + 1) * (CHUNK_A // 16)],
            add_ap=ones_sb[:].rearrange("p (j e) -> p j e", e=2),
            channels=128, num_elems=NE, d=2, num_idxs=CHUNK_A,
        ).then_inc(scat, 1); ns += 1
    ve.wait_ge(scat, ns)
    ve.reduce_sum(
        cntf[:],
        tbl[:].rearrange("p (r k e) -> p k e r", r=R, k=K, e=2)[:],
        axis=mybir.AxisListType.X,
    ).then_inc(sem, 1); nv += 1

    # ---- collapse partitions with PE: master = sel.T @ {sums, counts} ----
    with (
        nc.psum_tensor([NPAIR, 400], mybir.dt.float32) as ps0,
        nc.psum_tensor([NPAIR, 400], mybir.dt.float32) as ps1,
    ):
        for half, src in ((0, sumsf), (1, cntf)):
            ve.wait_ge(sem, nv)
            ve.tensor_copy(red_bf[:], src[:]).then_inc(sem, 1); nv += 1
            pe.wait_ge(sem, nv)
            pe.matmul(ps0[:], sel_sb[:], red_bf[:, 0:400], start=True, stop=True)
            pe.matmul(ps1[:], sel_sb[:], red_bf[:, 400:800], start=True, stop=True).then_inc(sem, 1); nv += 1
            act.wait_ge(sem, nv)
            act.copy(out_sb[0:NPAIR, half * 800 : half * 800 + 400], ps0[:])
            act.copy(out_sb[0:NPAIR, half * 800 + 400 : half * 800 + 800], ps1[:]).then_inc(sem, 1); nv += 1
        sp.wait_ge(sem, nv)
        sp.dma_start(master_d[0:NPAIR, :], out_sb[0:NPAIR, :]).then_inc(sem, 16); nv += 16
        sp.wait_ge(sem, nv)
    nc.compile()
    return nc


def _build_phaseB():
    nc = bacc.Bacc("TRN2", target_bir_lowering=False, debug=False, num_devices=8)
    # sums/cnt ship quad-interleaved: row q, col 4k+e = value for channel 4q+e
    sums_d = nc.dram_tensor("sums", [NQUAD, 1600], mybir.dt.float32, kind="ExternalInput")
    cnt_d = nc.dram_tensor("cnt", [NQUAD, 1600], mybir.dt.float32, kind="ExternalInput")
    idxB_d = nc.dram_tensor("idxB", [128, JQ8 // 16], mybir.dt.int16, kind="ExternalInput")
    out_d = nc.dram_tensor("outp", [128, JQ8 * 4], mybir.dt.float16, kind="ExternalOutput")
    mscr_d = nc.dram_tensor("mscr", [NQUAD, 1600], mybir.dt.float16)  # internal scratch

    sem = nc.alloc_semaphore("s")
    sp, gp, ve = nc.sync, nc.gpsimd, nc.vector

    sums_sb = nc.alloc_sbuf_tensor("sums_sb", [NQUAD, 1600], mybir.dt.float32)
    cnt_sb = nc.alloc_sbuf_tensor("cnt_sb", [NQUAD, 1600], mybir.dt.float32)
    means16 = nc.alloc_sbuf_tensor("means16", [NQUAD, 1600], mybir.dt.float16)
    tblB = nc.alloc_sbuf_tensor("tblB", [128, 1600], mybir.dt.float16)
    idxB_sb = nc.alloc_sbuf_tensor("idxB_sb", [128, JQ8 // 16], mybir.dt.int16)
    go_sb = [nc.alloc_sbuf_tensor(f"go{i}", [128, CHUNK_B * 4], mybir.dt.float16) for i in range(2)]

    nv = 0
    sp.dma_start(sums_sb[:], sums_d[:]).then_inc(sem, 16); nv += 16
    sp.dma_start(cnt_sb[:], cnt_d[:]).then_inc(sem, 16); nv += 16
    sp.dma_start(idxB_sb[:], idxB_d[:]).then_inc(sem, 16); nv += 16
    ve.wait_ge(sem, nv)
    ve.tensor_scalar(out=cnt_sb[:], in0=cnt_sb[:], scalar1=1.0, scalar2=None,
                     op0=mybir.AluOpType.max).then_inc(sem, 1); nv += 1
    ve.wait_ge(sem, nv)
    ve.reciprocal(cnt_sb[:], cnt_sb[:]).then_inc(sem, 1); nv += 1
    ve.wait_ge(sem, nv)
    ve.tensor_tensor(out=sums_sb[:], in0=sums_sb[:], in1=cnt_sb[:],
                     op=mybir.AluOpType.mult).then_inc(sem, 1); nv += 1
    ve.wait_ge(sem, nv)
    ve.tensor_copy(means16[:], sums_sb[:]).then_inc(sem, 1); nv += 1
    sp.wait_ge(sem, nv)
    sp.dma_start(mscr_d[:], means16[:]).then_inc(sem, 16); nv += 16
    # build the quad table: tblB[p=(g,q), (k e)] = mscr[q, (k e)], replicated per core g
    sp.wait_ge(sem, nv)
    for g in range(8):
        sp.dma_start(
            tblB[16 * g : 16 * g + 16, :],
            mscr_d[:],
        ).then_inc(sem, 16); nv += 16

    gp.wait_ge(sem, nv)
    base = nv
    gat = nc.alloc_semaphore("gat")
    ng = 0
    for cidx in range(NCHUNK_B8):
        buf = cidx % 2
        if cidx >= 2:
            gp.wait_ge(sem, base + (cidx - 1) * 16)
        gp.ap_gather(
            out_ap=go_sb[buf][:].rearrange("p (j e) -> p j e", e=4),
            in_ap=tblB[:].rearrange("p (k e) -> p k e", e=4),
            idxs_ap=idxB_sb[:, cidx * (CHUNK_B // 16) : (cidx + 1) * (CHUNK_B // 16)],
            channels=128, num_elems=400, d=4, num_idxs=CHUNK_B,
        ).then_inc(gat, 1); ng += 1
        sp.wait_ge(gat, ng)
        sp.dma_start(out_d[:, cidx * CHUNK_B * 4 : (cidx + 1) * CHUNK_B * 4], go_sb[buf][:]).then_inc(sem, 16)
    sp.wait_ge(sem, base + NCHUNK_B8 * 16)
    nc.compile()
    return nc


def _get_ncs():
    if "A" not in _CACHE:
        _CACHE["A"] = _build_phaseA()
    if "B" not in _CACHE:
        _CACHE["B"] = _build_phaseB()
    return _CACHE["A"], _CACHE["B"]


_SEL = None


def _sel_matrix():
    global _SEL
    if _SEL is None:
        s = np.zeros((128, NPAIR), dtype=_BF16)
        for p in range(128):
            a, _ = _pal(p)
            s[p, a] = 1.0
        _SEL = s
    return _SEL


_SLOT = None


def _slot_offsets():
    global _SLOT
    if _SLOT is None:
        _SLOT = ((np.arange(JQ) % R) * K).astype(np.int64)
    return _SLOT


def _prep_A(feat_half, idx_half):
    """feat_half [64, NH] f32, idx_half [NH] -> phase A inputs."""
    # partition p = (s, b, c16): a = s*16 + c16; channel = 2a + e
    addv = np.empty((2, NBLK, 16, JQ, 2), dtype=_BF16)  # [s, b, c16, j, e]
    fr = feat_half.astype(_BF16).reshape(2, 16, 2, NBLK, JQ)  # [s, c16, e, b, j]
    addv[:] = fr.transpose(0, 3, 1, 4, 2)  # -> [s, b, c16, j, e]
    idxw = np.empty((2, NBLK, 16, JQ // 16), dtype=np.int16)
    slot = _slot_offsets()
    for b in range(NBLK):
        ie = (idx_half[b * JQ : (b + 1) * JQ] + slot).astype(np.int16)
        w = ie.reshape(-1, 16).T  # [16, JQ//16]
        idxw[0, b] = w
        idxw[1, b] = w
    return {
        "addv": addv.reshape(128, JQ * 2),
        "idxA": idxw.reshape(128, JQ // 16),
        "sel": _sel_matrix(),
    }


def _prep_B(idx_half):
    # phase B partitions: p = (g, q): core g handles block g (NH/8 pixels)
    idxw = np.empty((8, 16, JQ8 // 16), dtype=np.int16)
    for g in range(8):
        w = idx_half[g * JQ8 : (g + 1) * JQ8].astype(np.int16).reshape(-1, 16).T
        idxw[g] = w
    return idxw.reshape(128, JQ8 // 16)


def _unpack_master(master):
    """[64, 1600] -> (sums_pair [32, 800] f32, counts [400] f32)."""
    return master[0:NPAIR, 0:800], master[0, 800:1600].reshape(400, 2)[:, 0]


def _unpack_out(buf):
    """[128, JQ8*4] fp16 -> [64, NH] f32. p=(g,q); out[4q+e, g*JQ8+j] = buf[p, 4j+e]."""
    v = buf.reshape(8, 16, JQ8, 4)               # [g, q, j, e]
    v = v.transpose(1, 3, 0, 2)                  # [q, e, g, j]
    return v.reshape(C, NH).astype(np.float32)


def kernel(features, spixel_idx):
    """features [4, 64, 262144] f32; spixel_idx [4, 262144] int -> [4, 64, 262144] f32."""
    global LAST_HW_NS
    import time as _time

    features = np.asarray(features)
    spixel_idx = np.asarray(spixel_idx)
    ncA, ncB = _get_ncs()

    in_maps_A = []
    idx_halves = []
    for core in range(8):
        b, h = core // 2, core % 2
        feat_half = features[b][:, h * NH : (h + 1) * NH]
        idx_half = np.asarray(spixel_idx[b][h * NH : (h + 1) * NH], dtype=np.int64)
        idx_halves.append(idx_half)
        in_maps_A.append(_prep_A(feat_half, idx_half))

    t0 = _time.time()
    resA = run_bass_kernel_spmd(ncA, in_maps_A, core_ids=list(range(8)))
    tA = _time.time() - t0

    in_maps_B = []
    for core in range(8):
        b = core // 2
        s0, c0 = _unpack_master(resA.results[2 * b]["master"])
        s1, c1 = _unpack_master(resA.results[2 * b + 1]["master"])
        sums_pair = s0 + s1                      # [32, 400, 2] viewed flat [32, 800]
        counts = c0 + c1
        # pair-interleaved [a, k, e2] -> quad-interleaved [q, k, e4]:
        # channel 4q+e4 = 2a+e2 with a = 2q + e4//2, e2 = e4%2
        sp3 = sums_pair.reshape(NQUAD, 2, 400, 2)        # [q, ahi, k, e2]
        sums_quad = np.ascontiguousarray(sp3.transpose(0, 2, 1, 3)).reshape(NQUAD, 1600)
        cnt_quad = np.ascontiguousarray(
            np.broadcast_to(np.repeat(counts, 4)[None, :], (NQUAD, 1600))
        ).astype(np.float32)
        in_maps_B.append({
            "sums": sums_quad,
            "cnt": cnt_quad,
            "idxB": _prep_B(idx_halves[core]),
        })

    t1 = _time.time()
    resB = run_bass_kernel_spmd(ncB, in_maps_B, core_ids=list(range(8)))
    tB = _time.time() - t1
    LAST_HW_NS = int((tA + tB) * 1e9)

    out = np.empty((B, C, N), dtype=np.float32)
    for core in range(8):
        b, h = core // 2, core % 2
        out[b][:, h * NH : (h + 1) * NH] = _unpack_out(resB.results[core]["outp"])
    return out


# revision 11
# speedup vs baseline: 2.2702x; 1.0371x over previous
"""MeanFeatureGather (per-segment mean + gather back) on 8 Trainium2 NeuronCores.

Sharding: 8 cores = 4 images (batch) x 2 half-images; each half-image is
processed channel-pair-major: SBUF partition p covers channel pair
a(p) = (p//64)*16 + p%16 and pixel block b(p) = (p//16)%4 (quarter of the
half-image), so all 8 GPSIMD Q7 cores work in parallel.

Launch A (per core): segment sums via the GPSIMD scatter_add ucode op
  (bf16, d=2 channel-pair payload, 32-way replica-slot rotation to defeat
  the ucode's pipelined read-modify-write hazard on duplicate indices),
  then a separate ones-payload scatter pass for the counts, DVE replica
  reductions, and a PE matmul that collapses partitions into a small
  [64, 1600] (sums, counts) table per core.
Host: pairwise adds the two half-image tables of each image (shard combine).
Launch B (per core): divides to per-segment means (DVE), packs an fp16
  channel-pair gather table, and gathers means to all pixels with the
  GPSIMD ap_gather ucode op (fp16, d=2 -> both channels of a pair per
  index), streaming fp16 results out; the host unpacks to [B, C, N] f32.
"""

import sys

sys.path.insert(0, "/opt/trn_rl_repo")

import numpy as np
import ml_dtypes

import concourse.bass as bass
import concourse.bacc as bacc
from concourse import mybir
from concourse.bass_utils import run_bass_kernel_spmd

B, C, N, K = 4, 64, 512 * 512, 400
NH = N // 2              # pixels per core (half image)          131072
R = 32                   # replica slots (scatter hazard window)
NE = K * R               # scatter table entries per partition    12800
NQUAD = C // 4           # channel quads                          16
JQ8 = NH // 8            # pixels per q7-core stream (8 blocks)    16384
CHUNK_A = 4096           # idx per feature scatter_add call
NCHUNK_A = JQ8 // CHUNK_A   # 4
CHUNK_ONE = 2048         # idx per counts scatter_add call
NCHUNK_ONE = JQ8 // CHUNK_ONE  # 8
CHUNK_B = 8192           # idx per ap_gather call
NCHUNK_B8 = JQ8 // CHUNK_B  # 2

_CACHE = {}
LAST_HW_NS = None

_BF16 = ml_dtypes.bfloat16
_FP16 = np.float16


def _pal(p):
    """partition -> (pair a, block b). g = p//16: a = (g//4)*16 + p%16, b = g%4."""
    g = p // 16
    return (g // 4) * 16 + p % 16, g % 4


def _build_phaseA():
    nc = bacc.Bacc("TRN2", target_bir_lowering=False, debug=False, num_devices=8)
    addv_d = nc.dram_tensor("addv", [128, JQ8 * 4], mybir.dt.bfloat16, kind="ExternalInput")
    idxA_d = nc.dram_tensor("idxA", [128, JQ8 // 16], mybir.dt.int16, kind="ExternalInput")
    sel_d = nc.dram_tensor("sel", [128, NQUAD], mybir.dt.bfloat16, kind="ExternalInput")
    master_d = nc.dram_tensor("master", [NQUAD, 3200], mybir.dt.float32, kind="ExternalOutput")

    sem = nc.alloc_semaphore("s")
    sp, gp, ve, pe, act = nc.sync, nc.gpsimd, nc.vector, nc.tensor, nc.scalar

    tbl = nc.alloc_sbuf_tensor("tbl", [128, NE * 4], mybir.dt.bfloat16)       # 102.4 KB
    sel_sb = nc.alloc_sbuf_tensor("sel_sb", [128, NQUAD], mybir.dt.bfloat16)
    idxA_sb = nc.alloc_sbuf_tensor("idxA_sb", [128, JQ8 // 16], mybir.dt.int16)  # 2 KB
    addv_sb = nc.alloc_sbuf_tensor("addv_sb", [128, CHUNK_A * 4], mybir.dt.bfloat16)  # 32 KB
    ones_sb = nc.alloc_sbuf_tensor("ones_sb", [128, CHUNK_ONE * 4], mybir.dt.bfloat16)  # 16 KB
    sumsf = nc.alloc_sbuf_tensor("sumsf", [128, 1600], mybir.dt.float32)
    cntf = nc.alloc_sbuf_tensor("cntf", [128, 1600], mybir.dt.float32)
    red_bf = nc.alloc_sbuf_tensor("red_bf", [128, 1600], mybir.dt.bfloat16)
    out_sb = nc.alloc_sbuf_tensor("out_sb", [NQUAD, 3200], mybir.dt.float32)

    nv = 0
    ve.memset(tbl[:], 0.0)
    ve.memset(ones_sb[:], 1.0).then_inc(sem, 1); nv += 1
    sp.dma_start(sel_sb[:], sel_d[:]).then_inc(sem, 16); nv += 16
    sp.dma_start(idxA_sb[:], idxA_d[:]).then_inc(sem, 16); nv += 16
    sp.dma_start(addv_sb[:], addv_d[:, 0 : CHUNK_A * 4]).then_inc(sem, 16); nv += 16

    scat = nc.alloc_semaphore("scat")
    ns = 0
    gp.wait_ge(sem, nv)
    # ---- feature scatter (channel quads, single buffer: load c, scatter c) ----
    for cidx in range(NCHUNK_A):
        if cidx >= 1:
            sp.wait_ge(scat, ns)
            sp.dma_start(addv_sb[:], addv_d[:, cidx * CHUNK_A * 4 : (cidx + 1) * CHUNK_A * 4]).then_inc(sem, 16); nv += 16
            gp.wait_ge(sem, nv)
        gp.scatter_add(
            in_ap=tbl[:].rearrange("p (k e) -> p k e", e=4),
            idxs_ap=idxA_sb[:, cidx * (CHUNK_A // 16) : (cidx + 1) * (CHUNK_A // 16)],
            add_ap=addv_sb[:].rearrange("p (j e) -> p j e", e=4),
            channels=128, num_elems=NE, d=4, num_idxs=CHUNK_A,
        ).then_inc(scat, 1); ns += 1

    # ---- reduce feature sums over replicas ----
    ve.wait_ge(scat, ns)
    ve.reduce_sum(
        sumsf[:],
        tbl[:].rearrange("p (r k e) -> p k e r", r=R, k=K, e=4)[:],
        axis=mybir.AxisListType.X,
    ).then_inc(sem, 1); nv += 1

    # ---- re-zero table, counts scatter with ones ----
    ve.memset(tbl[:], 0.0).then_inc(sem, 1); nv += 1
    gp.wait_ge(sem, nv)
    for cidx in range(NCHUNK_ONE):
        gp.scatter_add(
            in_ap=tbl[:].rearrange("p (k e) -> p k e", e=4),
            idxs_ap=idxA_sb[:, cidx * (CHUNK_ONE // 16) : (cidx + 1) * (CHUNK_ONE // 16)],
            add_ap=ones_sb[:].rearrange("p (j e) -> p j e", e=4),
            channels=128, num_elems=NE, d=4, num_idxs=CHUNK_ONE,
        ).then_inc(scat, 1); ns += 1
    ve.wait_ge(scat, ns)
    ve.reduce_sum(
        cntf[:],
        tbl[:].rearrange("p (r k e) -> p k e r", r=R, k=K, e=4)[:],
        axis=mybir.AxisListType.X,
    ).then_inc(sem, 1); nv += 1

    # ---- collapse partitions with PE: master = sel.T @ {sums, counts} ----
    with (
        nc.psum_tensor([NQUAD, 400], mybir.dt.float32) as ps0,
        nc.psum_tensor([NQUAD, 400], mybir.dt.float32) as ps1,
    ):
        for half, srcb in ((0, sumsf), (1, cntf)):
            ve.wait_ge(sem, nv)
            ve.tensor_copy(red_bf[:], srcb[:]).then_inc(sem, 1); nv += 1
            for m4 in range(0, 4, 2):
                pe.wait_ge(sem, nv)
                pe.matmul(ps0[:], sel_sb[:], red_bf[:, m4 * 400 : m4 * 400 + 400], start=True, stop=True)
                pe.matmul(ps1[:], sel_sb[:], red_bf[:, m4 * 400 + 400 : m4 * 400 + 800], start=True, stop=True).then_inc(sem, 1); nv += 1
                act.wait_ge(sem, nv)
                act.copy(out_sb[:, half * 1600 + m4 * 400 : half * 1600 + m4 * 400 + 400], ps0[:])
                act.copy(out_sb[:, half * 1600 + m4 * 400 + 400 : half * 1600 + m4 * 400 + 800], ps1[:]).then_inc(sem, 1); nv += 1
        sp.wait_ge(sem, nv)
        sp.dma_start(master_d[:], out_sb[:]).then_inc(sem, 16); nv += 16
        sp.wait_ge(sem, nv)
    nc.compile()
    return nc


def _build_phaseB():
    nc = bacc.Bacc("TRN2", target_bir_lowering=False, debug=False, num_devices=8)
    # sums/cnt ship quad-interleaved: row q, col 4k+e = value for channel 4q+e
    sums_d = nc.dram_tensor("sums", [NQUAD, 1600], mybir.dt.float32, kind="ExternalInput")
    cnt_d = nc.dram_tensor("cnt", [NQUAD, 1600], mybir.dt.float32, kind="ExternalInput")
    idxB_d = nc.dram_tensor("idxB", [128, JQ8 // 16], mybir.dt.int16, kind="ExternalInput")
    out_d = nc.dram_tensor("outp", [128, JQ8 * 4], mybir.dt.float16, kind="ExternalOutput")
    mscr_d = nc.dram_tensor("mscr", [NQUAD, 1600], mybir.dt.float16)  # internal scratch

    sem = nc.alloc_semaphore("s")
    sp, gp, ve = nc.sync, nc.gpsimd, nc.vector

    sums_sb = nc.alloc_sbuf_tensor("sums_sb", [NQUAD, 1600], mybir.dt.float32)
    cnt_sb = nc.alloc_sbuf_tensor("cnt_sb", [NQUAD, 1600], mybir.dt.float32)
    means16 = nc.alloc_sbuf_tensor("means16", [NQUAD, 1600], mybir.dt.float16)
    tblB = nc.alloc_sbuf_tensor("tblB", [128, 1600], mybir.dt.float16)
    idxB_sb = nc.alloc_sbuf_tensor("idxB_sb", [128, JQ8 // 16], mybir.dt.int16)
    go_sb = [nc.alloc_sbuf_tensor(f"go{i}", [128, CHUNK_B * 4], mybir.dt.float16) for i in range(2)]

    nv = 0
    sp.dma_start(sums_sb[:], sums_d[:]).then_inc(sem, 16); nv += 16
    sp.dma_start(cnt_sb[:], cnt_d[:]).then_inc(sem, 16); nv += 16
    sp.dma_start(idxB_sb[:], idxB_d[:]).then_inc(sem, 16); nv += 16
    ve.wait_ge(sem, nv)
    ve.tensor_scalar(out=cnt_sb[:], in0=cnt_sb[:], scalar1=1.0, scalar2=None,
                     op0=mybir.AluOpType.max).then_inc(sem, 1); nv += 1
    ve.wait_ge(sem, nv)
    ve.reciprocal(cnt_sb[:], cnt_sb[:]).then_inc(sem, 1); nv += 1
    ve.wait_ge(sem, nv)
    ve.tensor_tensor(out=sums_sb[:], in0=sums_sb[:], in1=cnt_sb[:],
                     op=mybir.AluOpType.mult).then_inc(sem, 1); nv += 1
    ve.wait_ge(sem, nv)
    ve.tensor_copy(means16[:], sums_sb[:]).then_inc(sem, 1); nv += 1
    sp.wait_ge(sem, nv)
    sp.dma_start(mscr_d[:], means16[:]).then_inc(sem, 16); nv += 16
    # build the quad table: tblB[p=(g,q), (k e)] = mscr[q, (k e)], replicated per core g
    sp.wait_ge(sem, nv)
    for g in range(8):
        sp.dma_start(
            tblB[16 * g : 16 * g + 16, :],
            mscr_d[:],
        ).then_inc(sem, 16); nv += 16

    gp.wait_ge(sem, nv)
    base = nv
    gat = nc.alloc_semaphore("gat")
    ng = 0
    for cidx in range(NCHUNK_B8):
        buf = cidx % 2
        if cidx >= 2:
            gp.wait_ge(sem, base + (cidx - 1) * 16)
        gp.ap_gather(
            out_ap=go_sb[buf][:].rearrange("p (j e) -> p j e", e=4),
            in_ap=tblB[:].rearrange("p (k e) -> p k e", e=4),
            idxs_ap=idxB_sb[:, cidx * (CHUNK_B // 16) : (cidx + 1) * (CHUNK_B // 16)],
            channels=128, num_elems=400, d=4, num_idxs=CHUNK_B,
        ).then_inc(gat, 1); ng += 1
        sp.wait_ge(gat, ng)
        sp.dma_start(out_d[:, cidx * CHUNK_B * 4 : (cidx + 1) * CHUNK_B * 4], go_sb[buf][:]).then_inc(sem, 16)
    sp.wait_ge(sem, base + NCHUNK_B8 * 16)
    nc.compile()
    return nc


def _get_ncs():
    if "A" not in _CACHE:
        _CACHE["A"] = _build_phaseA()
    if "B" not in _CACHE:
        _CACHE["B"] = _build_phaseB()
    return _CACHE["A"], _CACHE["B"]


_SEL = None


def _sel_matrix():
    global _SEL
    if _SEL is None:
        s = np.zeros((128, NQUAD), dtype=_BF16)
        for p in range(128):
            s[p, p % 16] = 1.0
        _SEL = s
    return _SEL


_SLOT = None


def _slot_offsets():
    global _SLOT
    if _SLOT is None:
        _SLOT = ((np.arange(JQ8) % R) * K).astype(np.int64)
    return _SLOT


def _prep_A(feat_half, idx_half):
    """feat_half [64, NH] f32, idx_half [NH] -> phase A inputs."""
    # partition p = (b, q): block b = p//16, quad q = p%16; channel = 4q + e
    addv = np.empty((8, 16, JQ8, 4), dtype=_BF16)  # [b, q, j, e]
    fr = feat_half.astype(_BF16).reshape(16, 4, 8, JQ8)  # [q, e, b, j]
    addv[:] = fr.transpose(2, 0, 3, 1)  # -> [b, q, j, e]
    idxw = np.empty((8, 16, JQ8 // 16), dtype=np.int16)
    slot = _slot_offsets()
    for b in range(8):
        ie = (idx_half[b * JQ8 : (b + 1) * JQ8] + slot).astype(np.int16)
        idxw[b] = ie.reshape(-1, 16).T  # [16, JQ8//16]
    return {
        "addv": addv.reshape(128, JQ8 * 4),
        "idxA": idxw.reshape(128, JQ8 // 16),
        "sel": _sel_matrix(),
    }


def _prep_B(idx_half):
    # phase B partitions: p = (g, q): core g handles block g (NH/8 pixels)
    idxw = np.empty((8, 16, JQ8 // 16), dtype=np.int16)
    for g in range(8):
        w = idx_half[g * JQ8 : (g + 1) * JQ8].astype(np.int16).reshape(-1, 16).T
        idxw[g] = w
    return idxw.reshape(128, JQ8 // 16)


def _unpack_master(master):
    """[16, 3200] -> (sums_quad [16, 1600] f32, counts [400] f32)."""
    return master[:, 0:1600], master[0, 1600:3200].reshape(400, 4)[:, 0]


def _unpack_out(buf):
    """[128, JQ8*4] fp16 -> [64, NH] f32. p=(g,q); out[4q+e, g*JQ8+j] = buf[p, 4j+e]."""
    v = buf.reshape(8, 16, JQ8, 4)               # [g, q, j, e]
    v = v.transpose(1, 3, 0, 2)                  # [q, e, g, j]
    return v.reshape(C, NH).astype(np.float32)


def kernel(features, spixel_idx):
    """features [4, 64, 262144] f32; spixel_idx [4, 262144] int -> [4, 64, 262144] f32."""
    global LAST_HW_NS
    import time as _time

    features = np.asarray(features)
    spixel_idx = np.asarray(spixel_idx)
    ncA, ncB = _get_ncs()

    in_maps_A = []
    idx_halves = []
    for core in range(8):
        b, h = core // 2, core % 2
        feat_half = features[b][:, h * NH : (h + 1) * NH]
        idx_half = np.asarray(spixel_idx[b][h * NH : (h + 1) * NH], dtype=np.int64)
        idx_halves.append(idx_half)
        in_maps_A.append(_prep_A(feat_half, idx_half))

    t0 = _time.time()
    resA = run_bass_kernel_spmd(ncA, in_maps_A, core_ids=list(range(8)))
    tA = _time.time() - t0

    in_maps_B = []
    for core in range(8):
        b = core // 2
        s0, c0 = _unpack_master(resA.results[2 * b]["master"])
        s1, c1 = _unpack_master(resA.results[2 * b + 1]["master"])
        sums_quad = np.ascontiguousarray(s0 + s1)        # [16, 1600], quad-interleaved
        counts = c0 + c1
        cnt_quad = np.ascontiguousarray(
            np.broadcast_to(np.repeat(counts, 4)[None, :], (NQUAD, 1600))
        ).astype(np.float32)
        in_maps_B.append({
            "sums": sums_quad,
            "cnt": cnt_quad,
            "idxB": _prep_B(idx_halves[core]),
        })

    t1 = _time.time()
    resB = run_bass_kernel_spmd(ncB, in_maps_B, core_ids=list(range(8)))
    tB = _time.time() - t1
    LAST_HW_NS = int((tA + tB) * 1e9)

    out = np.empty((B, C, N), dtype=np.float32)
    for core in range(8):
        b, h = core // 2, core % 2
        out[b][:, h * NH : (h + 1) * NH] = _unpack_out(resB.results[core]["outp"])
    return out
